# revision 2
# baseline (speedup 1.0000x reference)
"""BinaryLinear Trainium2 kernel.

Computes y = x @ (sign(W) * scale[:, None]).T + bias for
x [131072, 256] f32, W [256, 256] f32, scale/bias [256] f32.

Strategy: data-parallel across 8 NeuronCores — each core handles a
16384-row shard of x; W/scale/bias are replicated. Per core:

  prep (once):
    swT[i, o] = sign(W[o, i]) * scale[o]   built on-device:
      load W as two [128(o), 256(i)] tiles, ACT Sign, ACT scale-mul
      (scale is per-partition there), then 4 PE transposes into
      swT chunks [128(i), 256(o)].
    bias_bc [128, 256] = bias broadcast across partitions (stride-0 DMA).
  per 128-row tile (128 tiles):
    DMA x[b:b+128, :256] -> SBUF (contiguous rows)
    PE-transpose both [128,128] chunks (fp32 has no DMA transpose)
    2 accumulating fp32 matmuls: psum[b,o] += xT_chunk.T @ swT_chunk
    DVE: y = psum + bias_bc  (PSUM->SBUF eviction fused with bias add)
    DMA y tile out.

Scale is folded into the weights: sign(W)*scale is exactly representable
in fp32 (the reference computes the same product), so results match the
fp32 reference up to accumulation order.
"""

from contextlib import ExitStack

import numpy as np

import concourse.bass as bass
import concourse.tile as tile
from concourse import bacc, mybir
from concourse import bass_utils
from concourse.masks import make_identity

F32 = mybir.dt.float32
AF = mybir.ActivationFunctionType

B_FULL = 131072
I_DIM = 256
O_DIM = 256
N_CORES = 8
P = 128


def build_kernel(b_rows: int, mm_dtype=F32):
    """Build + compile the per-core Bass program for a b_rows-row shard."""
    assert b_rows % P == 0
    ntiles = b_rows // P

    nc = bacc.Bacc("TRN2", target_bir_lowering=False, debug=False)
    x_d = nc.dram_tensor("x", [b_rows, I_DIM], F32, kind="ExternalInput").ap()
    w_d = nc.dram_tensor("w", [O_DIM, I_DIM], F32, kind="ExternalInput").ap()
    scale_d = nc.dram_tensor("scale", [O_DIM], F32, kind="ExternalInput").ap()
    bias_d = nc.dram_tensor("bias", [O_DIM], F32, kind="ExternalInput").ap()
    y_d = nc.dram_tensor("y", [b_rows, O_DIM], F32, kind="ExternalOutput").ap()

    with tile.TileContext(nc) as tc, ExitStack() as ctx:
        _emit(ctx, tc, y_d, x_d, w_d, scale_d, bias_d, ntiles, mm_dtype)

    nc.compile()
    return nc


def _emit(ctx, tc, y, x, w, scale, bias, ntiles, mm_dtype):
    nc = tc.nc

    singles = ctx.enter_context(tc.tile_pool(name="singles", bufs=1))
    xpool = ctx.enter_context(tc.tile_pool(name="xin", bufs=4))
    xtpool = ctx.enter_context(tc.tile_pool(name="xt", bufs=4))
    ypool = ctx.enter_context(tc.tile_pool(name="yout", bufs=4))
    psum_t = ctx.enter_context(tc.tile_pool(name="psum_t", bufs=4, space="PSUM"))
    psum_y = ctx.enter_context(tc.tile_pool(name="psum_y", bufs=4, space="PSUM"))

    ident = singles.tile([P, P], F32)
    make_identity(nc, ident)

    # ---- prep: swT[ic] = [128(i), 256(o)] with entries sign(W[o,i])*scale[o]
    w_t = w.rearrange("(c p) i -> c p i", c=2)          # [2, 128, 256]
    scale_t = scale.rearrange("(c p a) -> c p a", c=2, a=1)  # [2, 128, 1]
    swT = [singles.tile([P, O_DIM], F32, name=f"swT{ic}", tag=f"swT{ic}")
           for ic in range(2)]
    for oc in range(2):
        w_sb = singles.tile([P, I_DIM], F32, tag=f"w{oc}")
        nc.sync.dma_start(out=w_sb, in_=w_t[oc])
        s_sb = singles.tile([P, 1], F32, tag=f"s{oc}")
        nc.sync.dma_start(out=s_sb, in_=scale_t[oc])
        sg_sb = singles.tile([P, I_DIM], F32, tag=f"sg{oc}")
        nc.scalar.activation(sg_sb, w_sb, AF.Sign)
        sw_sb = singles.tile([P, I_DIM], F32, tag=f"sw{oc}")
        nc.scalar.mul(sw_sb, sg_sb, s_sb)  # per-partition scale[o] multiply
        for ic in range(2):
            pt = psum_t.tile([P, P], F32, tag="psum_tr")
            nc.tensor.transpose(pt, sw_sb[:, ic * P:(ic + 1) * P], ident)
            nc.vector.tensor_copy(out=swT[ic][:, oc * P:(oc + 1) * P], in_=pt)

    # bias broadcast across all 128 partitions via stride-0 DMA
    bias_bc = singles.tile([P, O_DIM], F32)
    bias_rep = bass.AP(tensor=bias.tensor, offset=bias.offset,
                       ap=[[0, P]] + list(bias.ap))
    nc.sync.dma_start(out=bias_bc, in_=bias_rep)

    # ---- main loop over 128-row tiles
    x3 = x.rearrange("(n p) i -> n p i", p=P)
    y3 = y.rearrange("(n p) o -> n p o", p=P)
    for n in range(ntiles):
        x_sb = xpool.tile([P, I_DIM], F32, tag="x")
        nc.sync.dma_start(out=x_sb, in_=x3[n])

        xT = xtpool.tile([P, 2, P], F32, tag="xT")  # [i, chunk, b]
        for ic in range(2):
            pt = psum_t.tile([P, P], F32, tag="psum_tr")
            nc.tensor.transpose(pt, x_sb[:, ic * P:(ic + 1) * P], ident)
            nc.vector.tensor_copy(out=xT[:, ic], in_=pt)

        py = psum_y.tile([P, O_DIM], F32, tag="py")
        for ic in range(2):
            nc.tensor.matmul(py, lhsT=xT[:, ic], rhs=swT[ic],
                             start=(ic == 0), stop=(ic == 1))

        y_sb = ypool.tile([P, O_DIM], F32, tag="y")
        nc.vector.tensor_add(out=y_sb, in0=py, in1=bias_bc)
        nc.sync.dma_start(out=y3[n], in_=y_sb)


_CACHE = {}


def _get_nc(b_rows):
    if b_rows not in _CACHE:
        _CACHE[b_rows] = build_kernel(b_rows)
    return _CACHE[b_rows]


def run_sharded(x, W, scale, bias, trace=False):
    """Run the SPMD kernel on 8 cores; returns (y_full, BassKernelResults)."""
    x = np.ascontiguousarray(x, dtype=np.float32)
    W = np.ascontiguousarray(W, dtype=np.float32)
    scale = np.ascontiguousarray(scale, dtype=np.float32)
    bias = np.ascontiguousarray(bias, dtype=np.float32)
    b_shard = x.shape[0] // N_CORES
    nc = _get_nc(b_shard)
    xs = x.reshape(N_CORES, b_shard, I_DIM)
    in_maps = [
        {"x": np.ascontiguousarray(xs[c]), "w": W, "scale": scale, "bias": bias}
        for c in range(N_CORES)
    ]
    res = bass_utils.run_bass_kernel_spmd(
        nc, in_maps, core_ids=list(range(N_CORES)), trace=trace,
        trace_cores=list(range(N_CORES)) if trace else None,
    )
    y = np.concatenate([res.results[c]["y"] for c in range(N_CORES)], axis=0)
    return y, res


def kernel(x, W, scale, bias):
    y, _ = run_sharded(x, W, scale, bias, trace=False)
    return y


# revision 9
# speedup vs baseline: 1.0671x; 1.0671x over previous
"""BinaryLinear Trainium2 kernel.

Computes y = x @ (sign(W) * scale[:, None]).T + bias for
x [131072, 256] f32, W [256, 256] f32, scale/bias [256] f32.

Strategy: data-parallel across 8 NeuronCores — each core handles a
16384-row shard of x; W/scale/bias are replicated. Per core:

  prep (once):
    swT[i, o] = sign(W[o, i]) * scale[o]   built on-device:
      load W as two [128(o), 256(i)] tiles, ACT Sign, ACT scale-mul
      (scale is per-partition there), then 4 PE transposes into
      swT chunks [128(i), 256(o)].
    bias_bc [128, 256] = bias broadcast across partitions (stride-0 DMA).
  per 128-row tile (128 tiles):
    DMA x[b:b+128, :256] -> SBUF (contiguous rows)
    PE-transpose both [128,128] chunks (fp32 has no DMA transpose)
    2 accumulating fp32 matmuls: psum[b,o] += xT_chunk.T @ swT_chunk
    DVE: y = psum + bias_bc  (PSUM->SBUF eviction fused with bias add)
    DMA y tile out.

Scale is folded into the weights: sign(W)*scale is exactly representable
in fp32 (the reference computes the same product), so results match the
fp32 reference up to accumulation order.
"""

from contextlib import ExitStack

import numpy as np

import concourse.bass as bass
import concourse.tile as tile
from concourse import bacc, mybir
from concourse import bass_utils
from concourse.masks import make_identity

F32 = mybir.dt.float32
AF = mybir.ActivationFunctionType

B_FULL = 131072
I_DIM = 256
O_DIM = 256
N_CORES = 8
P = 128


def build_kernel(b_rows: int, mm_dtype=F32):
    """Build + compile the per-core Bass program for a b_rows-row shard."""
    assert b_rows % P == 0
    ntiles = b_rows // P

    nc = bacc.Bacc("TRN2", target_bir_lowering=False, debug=False)
    x_d = nc.dram_tensor("x", [b_rows, I_DIM], F32, kind="ExternalInput").ap()
    w_d = nc.dram_tensor("w", [O_DIM, I_DIM], F32, kind="ExternalInput").ap()
    scale_d = nc.dram_tensor("scale", [O_DIM], F32, kind="ExternalInput").ap()
    bias_d = nc.dram_tensor("bias", [O_DIM], F32, kind="ExternalInput").ap()
    y_d = nc.dram_tensor("y", [b_rows, O_DIM], F32, kind="ExternalOutput").ap()

    with tile.TileContext(nc) as tc, ExitStack() as ctx:
        _emit(ctx, tc, y_d, x_d, w_d, scale_d, bias_d, ntiles, mm_dtype)

    nc.compile()
    return nc


def _emit(ctx, tc, y, x, w, scale, bias, ntiles, mm_dtype):
    nc = tc.nc

    singles = ctx.enter_context(tc.tile_pool(name="singles", bufs=1))
    xpool = ctx.enter_context(tc.tile_pool(name="xin", bufs=4))
    xtpool = ctx.enter_context(tc.tile_pool(name="xt", bufs=4))
    ypool = ctx.enter_context(tc.tile_pool(name="yout", bufs=4))
    psum_t = ctx.enter_context(tc.tile_pool(name="psum_t", bufs=4, space="PSUM"))
    psum_y = ctx.enter_context(tc.tile_pool(name="psum_y", bufs=4, space="PSUM"))

    ident = singles.tile([P, P], F32)
    make_identity(nc, ident)

    # ---- prep: swT[ic] = [128(i), 256(o)] with entries sign(W[o,i])*scale[o]
    w_t = w.rearrange("(c p) i -> c p i", c=2)          # [2, 128, 256]
    scale_t = scale.rearrange("(c p a) -> c p a", c=2, a=1)  # [2, 128, 1]
    swT = [singles.tile([P, O_DIM], mm_dtype, name=f"swT{ic}", tag=f"swT{ic}")
           for ic in range(2)]
    for oc in range(2):
        w_sb = singles.tile([P, I_DIM], F32, tag=f"w{oc}")
        nc.sync.dma_start(out=w_sb, in_=w_t[oc])
        s_sb = singles.tile([P, 1], F32, tag=f"s{oc}")
        nc.sync.dma_start(out=s_sb, in_=scale_t[oc])
        sg_sb = singles.tile([P, I_DIM], F32, tag=f"sg{oc}")
        nc.scalar.activation(sg_sb, w_sb, AF.Sign)
        sw_sb = singles.tile([P, I_DIM], F32, tag=f"sw{oc}")
        nc.scalar.mul(sw_sb, sg_sb, s_sb)  # per-partition scale[o] multiply
        for ic in range(2):
            pt = psum_t.tile([P, P], F32, tag="psum_tr")
            nc.tensor.transpose(pt, sw_sb[:, ic * P:(ic + 1) * P], ident)
            nc.vector.tensor_copy(out=swT[ic][:, oc * P:(oc + 1) * P], in_=pt)

    # bias broadcast across all 128 partitions via stride-0 DMA
    bias_bc = singles.tile([P, O_DIM], F32)
    bias_rep = bass.AP(tensor=bias.tensor, offset=bias.offset,
                       ap=[[0, P]] + list(bias.ap))
    nc.sync.dma_start(out=bias_bc, in_=bias_rep)

    # ---- main loop over 128-row tiles
    x3 = x.rearrange("(n p) i -> n p i", p=P)
    y3 = y.rearrange("(n p) o -> n p o", p=P)
    for n in range(ntiles):
        x_sb = xpool.tile([P, I_DIM], F32, tag="x")
        nc.sync.dma_start(out=x_sb, in_=x3[n])

        xT = xtpool.tile([P, 2, P], mm_dtype, tag="xT")  # [i, chunk, b]
        for ic in range(2):
            pt = psum_t.tile([P, P], F32, tag="psum_tr")
            nc.tensor.transpose(pt, x_sb[:, ic * P:(ic + 1) * P], ident)
            nc.vector.tensor_copy(out=xT[:, ic], in_=pt)

        py = psum_y.tile([P, O_DIM], F32, tag="py")
        for ic in range(2):
            nc.tensor.matmul(py, lhsT=xT[:, ic], rhs=swT[ic],
                             start=(ic == 0), stop=(ic == 1))

        y_sb = ypool.tile([P, O_DIM], F32, tag="y")
        nc.vector.tensor_add(out=y_sb, in0=py, in1=bias_bc)
        nc.sync.dma_start(out=y3[n], in_=y_sb)


_CACHE = {}


def _get_nc(b_rows, mm_dtype=F32):
    key = (b_rows, str(mm_dtype))
    if key not in _CACHE:
        _CACHE[key] = build_kernel(b_rows, mm_dtype)
    return _CACHE[key]


def run_sharded(x, W, scale, bias, trace=False, mm_dtype=F32):
    """Run the SPMD kernel on 8 cores; returns (y_full, BassKernelResults)."""
    x = np.ascontiguousarray(x, dtype=np.float32)
    W = np.ascontiguousarray(W, dtype=np.float32)
    scale = np.ascontiguousarray(scale, dtype=np.float32)
    bias = np.ascontiguousarray(bias, dtype=np.float32)
    b_shard = x.shape[0] // N_CORES
    nc = _get_nc(b_shard, mm_dtype)
    xs = x.reshape(N_CORES, b_shard, I_DIM)
    in_maps = [
        {"x": np.ascontiguousarray(xs[c]), "w": W, "scale": scale, "bias": bias}
        for c in range(N_CORES)
    ]
    res = bass_utils.run_bass_kernel_spmd(
        nc, in_maps, core_ids=list(range(N_CORES)), trace=trace,
        trace_cores=list(range(N_CORES)) if trace else None,
    )
    y = np.concatenate([res.results[c]["y"] for c in range(N_CORES)], axis=0)
    return y, res


def kernel(x, W, scale, bias):
    y, _ = run_sharded(x, W, scale, bias, trace=False)
    return y


# revision 12
# speedup vs baseline: 1.3574x; 1.2721x over previous
"""BinaryLinear Trainium2 kernel.

Computes y = x @ (sign(W) * scale[:, None]).T + bias for
x [131072, 256] f32, W [256, 256] f32, scale/bias [256] f32.

Strategy: data-parallel across 8 NeuronCores — each core handles a
16384-row shard of x; W/scale/bias are replicated. Per core:

  prep (once):
    swT[i, o] = sign(W[o, i]) * scale[o]   built on-device:
      load W as two [128(o), 256(i)] tiles, ACT Sign, ACT scale-mul
      (scale is per-partition there), then 4 PE transposes into
      swT chunks [128(i), 256(o)].
    bias_bc [128, 256] = bias broadcast across partitions (stride-0 DMA).
  per 128-row tile (128 tiles):
    DMA x[b:b+128, :256] -> SBUF (contiguous rows)
    PE-transpose both [128,128] chunks (fp32 has no DMA transpose)
    2 accumulating fp32 matmuls: psum[b,o] += xT_chunk.T @ swT_chunk
    DVE: y = psum + bias_bc  (PSUM->SBUF eviction fused with bias add)
    DMA y tile out.

Scale is folded into the weights: sign(W)*scale is exactly representable
in fp32 (the reference computes the same product), so results match the
fp32 reference up to accumulation order.
"""

from contextlib import ExitStack

import numpy as np

import concourse.bass as bass
import concourse.tile as tile
from concourse import bacc, mybir
from concourse import bass_utils
from concourse.masks import make_identity

F32 = mybir.dt.float32
AF = mybir.ActivationFunctionType

B_FULL = 131072
I_DIM = 256
O_DIM = 256
N_CORES = 8
P = 128


def build_kernel(b_rows: int, mm_dtype=F32):
    """Build + compile the per-core Bass program for a b_rows-row shard."""
    assert b_rows % P == 0
    ntiles = b_rows // P

    nc = bacc.Bacc("TRN2", target_bir_lowering=False, debug=False)
    x_d = nc.dram_tensor("x", [b_rows, I_DIM], F32, kind="ExternalInput").ap()
    w_d = nc.dram_tensor("w", [O_DIM, I_DIM], F32, kind="ExternalInput").ap()
    scale_d = nc.dram_tensor("scale", [O_DIM], F32, kind="ExternalInput").ap()
    bias_d = nc.dram_tensor("bias", [O_DIM], F32, kind="ExternalInput").ap()
    y_d = nc.dram_tensor("y", [b_rows, O_DIM], F32, kind="ExternalOutput").ap()

    with tile.TileContext(nc) as tc, ExitStack() as ctx:
        _emit(ctx, tc, y_d, x_d, w_d, scale_d, bias_d, ntiles, mm_dtype)

    nc.compile()
    return nc


def _emit(ctx, tc, y, x, w, scale, bias, ntiles, mm_dtype):
    nc = tc.nc

    singles = ctx.enter_context(tc.tile_pool(name="singles", bufs=1))
    xpool = ctx.enter_context(tc.tile_pool(name="xin", bufs=6))
    xtpool = ctx.enter_context(tc.tile_pool(name="xt", bufs=8))
    ypool = ctx.enter_context(tc.tile_pool(name="yout", bufs=6))
    psum_t = ctx.enter_context(tc.tile_pool(name="psum_t", bufs=4, space="PSUM"))
    psum_y = ctx.enter_context(tc.tile_pool(name="psum_y", bufs=4, space="PSUM"))

    ident = singles.tile([P, P], F32)
    make_identity(nc, ident)

    # ---- prep: swT[ic] = [128(i), 256(o)] with entries sign(W[o,i])*scale[o]
    w_t = w.rearrange("(c p) i -> c p i", c=2)          # [2, 128, 256]
    scale_t = scale.rearrange("(c p a) -> c p a", c=2, a=1)  # [2, 128, 1]
    swT = [singles.tile([P, O_DIM], mm_dtype, name=f"swT{ic}", tag=f"swT{ic}")
           for ic in range(2)]
    for oc in range(2):
        w_sb = singles.tile([P, I_DIM], F32, tag=f"w{oc}")
        nc.sync.dma_start(out=w_sb, in_=w_t[oc])
        s_sb = singles.tile([P, 1], F32, tag=f"s{oc}")
        nc.sync.dma_start(out=s_sb, in_=scale_t[oc])
        sg_sb = singles.tile([P, I_DIM], F32, tag=f"sg{oc}")
        nc.scalar.activation(sg_sb, w_sb, AF.Sign)
        sw_sb = singles.tile([P, I_DIM], F32, tag=f"sw{oc}")
        nc.scalar.mul(sw_sb, sg_sb, s_sb)  # per-partition scale[o] multiply
        for ic in range(2):
            pt = psum_t.tile([P, P], F32, tag="psum_tr")
            nc.tensor.transpose(pt, sw_sb[:, ic * P:(ic + 1) * P], ident)
            nc.vector.tensor_copy(out=swT[ic][:, oc * P:(oc + 1) * P], in_=pt)

    # bias broadcast across all 128 partitions via stride-0 DMA
    bias_bc = singles.tile([P, O_DIM], F32)
    bias_rep = bass.AP(tensor=bias.tensor, offset=bias.offset,
                       ap=[[0, P]] + list(bias.ap))
    nc.sync.dma_start(out=bias_bc, in_=bias_rep)

    # ---- main loop: 2 row-tiles (256 rows) per DMA batch
    assert ntiles % 2 == 0
    x4 = x.rearrange("(n s p) i -> n p s i", p=P, s=2)
    y4 = y.rearrange("(n s p) o -> n p s o", p=P, s=2)
    for n in range(ntiles // 2):
        x_sb = xpool.tile([P, 2, I_DIM], F32, tag="x")
        nc.sync.dma_start(out=x_sb, in_=x4[n])

        y_sb = ypool.tile([P, 2, O_DIM], F32, tag="y")
        for s in range(2):
            xT = xtpool.tile([P, 2, P], mm_dtype, tag="xT")  # [i, chunk, b]
            for ic in range(2):
                pt = psum_t.tile([P, P], F32, tag="psum_tr")
                nc.tensor.transpose(pt, x_sb[:, s, ic * P:(ic + 1) * P], ident)
                nc.scalar.copy(out=xT[:, ic], in_=pt)

            py = psum_y.tile([P, O_DIM], F32, tag="py")
            for ic in range(2):
                nc.tensor.matmul(py, lhsT=xT[:, ic], rhs=swT[ic],
                                 start=(ic == 0), stop=(ic == 1))

            nc.vector.tensor_add(out=y_sb[:, s], in0=py, in1=bias_bc)
        nc.sync.dma_start(out=y4[n], in_=y_sb)


_CACHE = {}


def _get_nc(b_rows, mm_dtype=F32):
    key = (b_rows, str(mm_dtype))
    if key not in _CACHE:
        _CACHE[key] = build_kernel(b_rows, mm_dtype)
    return _CACHE[key]


def run_sharded(x, W, scale, bias, trace=False, mm_dtype=F32):
    """Run the SPMD kernel on 8 cores; returns (y_full, BassKernelResults)."""
    x = np.ascontiguousarray(x, dtype=np.float32)
    W = np.ascontiguousarray(W, dtype=np.float32)
    scale = np.ascontiguousarray(scale, dtype=np.float32)
    bias = np.ascontiguousarray(bias, dtype=np.float32)
    b_shard = x.shape[0] // N_CORES
    nc = _get_nc(b_shard, mm_dtype)
    xs = x.reshape(N_CORES, b_shard, I_DIM)
    in_maps = [
        {"x": np.ascontiguousarray(xs[c]), "w": W, "scale": scale, "bias": bias}
        for c in range(N_CORES)
    ]
    res = bass_utils.run_bass_kernel_spmd(
        nc, in_maps, core_ids=list(range(N_CORES)), trace=trace,
        trace_cores=list(range(N_CORES)) if trace else None,
    )
    y = np.concatenate([res.results[c]["y"] for c in range(N_CORES)], axis=0)
    return y, res


def kernel(x, W, scale, bias):
    y, _ = run_sharded(x, W, scale, bias, trace=False)
    return y


# revision 13
# speedup vs baseline: 1.5274x; 1.1253x over previous
"""BinaryLinear Trainium2 kernel.

Computes y = x @ (sign(W) * scale[:, None]).T + bias for
x [131072, 256] f32, W [256, 256] f32, scale/bias [256] f32.

Strategy: data-parallel across 8 NeuronCores — each core handles a
16384-row shard of x; W/scale/bias are replicated. Per core:

  prep (once):
    swT[i, o] = sign(W[o, i]) * scale[o]   built on-device:
      load W as two [128(o), 256(i)] tiles, ACT Sign, ACT scale-mul
      (scale is per-partition there), then 4 PE transposes into
      swT chunks [128(i), 256(o)].
    bias_bc [128, 256] = bias broadcast across partitions (stride-0 DMA).
  per 128-row tile (128 tiles):
    DMA x[b:b+128, :256] -> SBUF (contiguous rows)
    PE-transpose both [128,128] chunks (fp32 has no DMA transpose)
    2 accumulating fp32 matmuls: psum[b,o] += xT_chunk.T @ swT_chunk
    DVE: y = psum + bias_bc  (PSUM->SBUF eviction fused with bias add)
    DMA y tile out.

Scale is folded into the weights: sign(W)*scale is exactly representable
in fp32 (the reference computes the same product), so results match the
fp32 reference up to accumulation order.
"""

from contextlib import ExitStack

import numpy as np

import concourse.bass as bass
import concourse.tile as tile
from concourse import bacc, mybir
from concourse import bass_utils
from concourse.masks import make_identity

F32 = mybir.dt.float32
AF = mybir.ActivationFunctionType

B_FULL = 131072
I_DIM = 256
O_DIM = 256
N_CORES = 8
P = 128


def build_kernel(b_rows: int, mm_dtype=F32):
    """Build + compile the per-core Bass program for a b_rows-row shard."""
    assert b_rows % P == 0
    ntiles = b_rows // P

    nc = bacc.Bacc("TRN2", target_bir_lowering=False, debug=False)
    x_d = nc.dram_tensor("x", [b_rows, I_DIM], F32, kind="ExternalInput").ap()
    w_d = nc.dram_tensor("w", [O_DIM, I_DIM], F32, kind="ExternalInput").ap()
    scale_d = nc.dram_tensor("scale", [O_DIM], F32, kind="ExternalInput").ap()
    bias_d = nc.dram_tensor("bias", [O_DIM], F32, kind="ExternalInput").ap()
    y_d = nc.dram_tensor("y", [b_rows, O_DIM], F32, kind="ExternalOutput").ap()

    with tile.TileContext(nc) as tc, ExitStack() as ctx:
        _emit(ctx, tc, y_d, x_d, w_d, scale_d, bias_d, ntiles, mm_dtype)

    nc.compile()
    return nc


def _emit(ctx, tc, y, x, w, scale, bias, ntiles, mm_dtype):
    nc = tc.nc

    singles = ctx.enter_context(tc.tile_pool(name="singles", bufs=1))
    xpool = ctx.enter_context(tc.tile_pool(name="xin", bufs=6))
    xtpool = ctx.enter_context(tc.tile_pool(name="xt", bufs=8))
    ypool = ctx.enter_context(tc.tile_pool(name="yout", bufs=6))
    psum_t = ctx.enter_context(tc.tile_pool(name="psum_t", bufs=4, space="PSUM"))
    psum_y = ctx.enter_context(tc.tile_pool(name="psum_y", bufs=4, space="PSUM"))

    ident = singles.tile([P, P], F32)
    make_identity(nc, ident)

    # ---- prep: swT[ic] = [128(i), 256(o)] with entries sign(W[o,i])*scale[o]
    w_t = w.rearrange("(c p) i -> c p i", c=2)          # [2, 128, 256]
    scale_t = scale.rearrange("(c p a) -> c p a", c=2, a=1)  # [2, 128, 1]
    swT = [singles.tile([P, O_DIM], mm_dtype, name=f"swT{ic}", tag=f"swT{ic}")
           for ic in range(2)]
    for oc in range(2):
        w_sb = singles.tile([P, I_DIM], F32, tag=f"w{oc}")
        nc.sync.dma_start(out=w_sb, in_=w_t[oc])
        s_sb = singles.tile([P, 1], F32, tag=f"s{oc}")
        nc.sync.dma_start(out=s_sb, in_=scale_t[oc])
        sg_sb = singles.tile([P, I_DIM], F32, tag=f"sg{oc}")
        nc.scalar.activation(sg_sb, w_sb, AF.Sign)
        sw_sb = singles.tile([P, I_DIM], F32, tag=f"sw{oc}")
        nc.scalar.mul(sw_sb, sg_sb, s_sb)  # per-partition scale[o] multiply
        for ic in range(2):
            pt = psum_t.tile([P, P], F32, tag="psum_tr")
            nc.tensor.transpose(pt, sw_sb[:, ic * P:(ic + 1) * P], ident)
            nc.vector.tensor_copy(out=swT[ic][:, oc * P:(oc + 1) * P], in_=pt)

    # bias broadcast across all 128 partitions (x2 in free dim) via stride-0 DMA
    bias_bc = singles.tile([P, 2, O_DIM], F32)
    bias_rep = bass.AP(tensor=bias.tensor, offset=bias.offset,
                       ap=[[0, P], [0, 2]] + list(bias.ap))
    nc.sync.dma_start(out=bias_bc, in_=bias_rep)

    # ---- main loop: 4 row-tiles (512 rows) per DMA batch
    SB = 4  # row-tiles per DMA batch
    assert ntiles % SB == 0
    x4 = x.rearrange("(n s p) i -> n p s i", p=P, s=SB)
    y4 = y.rearrange("(n s p) o -> n p s o", p=P, s=SB)
    for n in range(ntiles // SB):
        x_sb = xpool.tile([P, SB, I_DIM], F32, tag="x")
        nc.sync.dma_start(out=x_sb, in_=x4[n])

        y_sb = ypool.tile([P, SB, O_DIM], F32, tag="y")
        for sp in range(SB // 2):  # psum_y bank holds 2 row-tiles
            py = psum_y.tile([P, 2, O_DIM], F32, tag="py")
            for s2 in range(2):
                s = sp * 2 + s2
                pt = psum_t.tile([P, 2, P], F32, tag="psum_tr")
                for ic in range(2):
                    nc.tensor.transpose(pt[:, ic], x_sb[:, s, ic * P:(ic + 1) * P],
                                        ident)
                xT = xtpool.tile([P, 2, P], mm_dtype, tag="xT")  # [i, chunk, b]
                nc.scalar.copy(out=xT, in_=pt)
                for ic in range(2):
                    nc.tensor.matmul(py[:, s2], lhsT=xT[:, ic], rhs=swT[ic],
                                     start=(ic == 0), stop=(ic == 1))
            nc.vector.tensor_add(out=y_sb[:, sp * 2:sp * 2 + 2], in0=py,
                                 in1=bias_bc)
        nc.sync.dma_start(out=y4[n], in_=y_sb)


_CACHE = {}


def _get_nc(b_rows, mm_dtype=F32):
    key = (b_rows, str(mm_dtype))
    if key not in _CACHE:
        _CACHE[key] = build_kernel(b_rows, mm_dtype)
    return _CACHE[key]


def run_sharded(x, W, scale, bias, trace=False, mm_dtype=F32):
    """Run the SPMD kernel on 8 cores; returns (y_full, BassKernelResults)."""
    x = np.ascontiguousarray(x, dtype=np.float32)
    W = np.ascontiguousarray(W, dtype=np.float32)
    scale = np.ascontiguousarray(scale, dtype=np.float32)
    bias = np.ascontiguousarray(bias, dtype=np.float32)
    b_shard = x.shape[0] // N_CORES
    nc = _get_nc(b_shard, mm_dtype)
    xs = x.reshape(N_CORES, b_shard, I_DIM)
    in_maps = [
        {"x": np.ascontiguousarray(xs[c]), "w": W, "scale": scale, "bias": bias}
        for c in range(N_CORES)
    ]
    res = bass_utils.run_bass_kernel_spmd(
        nc, in_maps, core_ids=list(range(N_CORES)), trace=trace,
        trace_cores=list(range(N_CORES)) if trace else None,
    )
    y = np.concatenate([res.results[c]["y"] for c in range(N_CORES)], axis=0)
    return y, res


def kernel(x, W, scale, bias):
    y, _ = run_sharded(x, W, scale, bias, trace=False)
    return y


# revision 15
# speedup vs baseline: 1.9046x; 1.2470x over previous
"""BinaryLinear Trainium2 kernel.

Computes y = x @ (sign(W) * scale[:, None]).T + bias for
x [131072, 256] f32, W [256, 256] f32, scale/bias [256] f32.

Strategy: data-parallel across 8 NeuronCores — each core handles a
16384-row shard of x; W/scale/bias are replicated. Per core:

  prep (once):
    swT[i, o] = sign(W[o, i]) * scale[o]   built on-device:
      load W as two [128(o), 256(i)] tiles, ACT Sign, ACT scale-mul
      (scale is per-partition there), then 4 PE transposes into
      swT chunks [128(i), 256(o)].
    bias_bc [128, 256] = bias broadcast across partitions (stride-0 DMA).
  per 128-row tile (128 tiles):
    DMA x[b:b+128, :256] -> SBUF (contiguous rows)
    PE-transpose both [128,128] chunks (fp32 has no DMA transpose)
    2 accumulating fp32 matmuls: psum[b,o] += xT_chunk.T @ swT_chunk
    DVE: y = psum + bias_bc  (PSUM->SBUF eviction fused with bias add)
    DMA y tile out.

Scale is folded into the weights: sign(W)*scale is exactly representable
in fp32 (the reference computes the same product), so results match the
fp32 reference up to accumulation order.
"""

from contextlib import ExitStack

import numpy as np

import concourse.bass as bass
import concourse.tile as tile
from concourse import bacc, mybir
from concourse import bass_utils
from concourse.masks import make_identity

F32 = mybir.dt.float32
AF = mybir.ActivationFunctionType

B_FULL = 131072
I_DIM = 256
O_DIM = 256
N_CORES = 8
P = 128


def build_kernel(b_rows: int, mm_dtype=F32):
    """Build + compile the per-core Bass program for a b_rows-row shard."""
    assert b_rows % P == 0
    ntiles = b_rows // P

    nc = bacc.Bacc("TRN2", target_bir_lowering=False, debug=False)
    x_d = nc.dram_tensor("x", [b_rows, I_DIM], F32, kind="ExternalInput").ap()
    w_d = nc.dram_tensor("w", [O_DIM, I_DIM], F32, kind="ExternalInput").ap()
    scale_d = nc.dram_tensor("scale", [O_DIM], F32, kind="ExternalInput").ap()
    bias_d = nc.dram_tensor("bias", [O_DIM], F32, kind="ExternalInput").ap()
    y_d = nc.dram_tensor("y", [b_rows, O_DIM], F32, kind="ExternalOutput").ap()

    with tile.TileContext(nc) as tc, ExitStack() as ctx:
        _emit(ctx, tc, y_d, x_d, w_d, scale_d, bias_d, ntiles, mm_dtype)

    nc.compile()
    return nc


def _emit(ctx, tc, y, x, w, scale, bias, ntiles, mm_dtype):
    nc = tc.nc

    singles = ctx.enter_context(tc.tile_pool(name="singles", bufs=1))
    xpool = ctx.enter_context(tc.tile_pool(name="xin", bufs=8))
    xtpool = ctx.enter_context(tc.tile_pool(name="xt", bufs=8))
    ypool = ctx.enter_context(tc.tile_pool(name="yout", bufs=6))
    psum_t = ctx.enter_context(tc.tile_pool(name="psum_t", bufs=4, space="PSUM"))
    psum_y = ctx.enter_context(tc.tile_pool(name="psum_y", bufs=4, space="PSUM"))

    ident = singles.tile([P, P], F32)
    make_identity(nc, ident)

    # ---- prep: swT[ic] = [128(i), 256(o)] with entries sign(W[o,i])*scale[o]
    w_t = w.rearrange("(c p) i -> c p i", c=2)          # [2, 128, 256]
    scale_t = scale.rearrange("(c p a) -> c p a", c=2, a=1)  # [2, 128, 1]
    swT = [singles.tile([P, O_DIM], mm_dtype, name=f"swT{ic}", tag=f"swT{ic}")
           for ic in range(2)]
    for oc in range(2):
        w_sb = singles.tile([P, I_DIM], F32, tag=f"w{oc}")
        nc.sync.dma_start(out=w_sb, in_=w_t[oc])
        s_sb = singles.tile([P, 1], F32, tag=f"s{oc}")
        nc.sync.dma_start(out=s_sb, in_=scale_t[oc])
        sg_sb = singles.tile([P, I_DIM], F32, tag=f"sg{oc}")
        nc.scalar.activation(sg_sb, w_sb, AF.Sign)
        sw_sb = singles.tile([P, I_DIM], F32, tag=f"sw{oc}")
        nc.scalar.mul(sw_sb, sg_sb, s_sb)  # per-partition scale[o] multiply
        for ic in range(2):
            pt = psum_t.tile([P, P], F32, tag="psum_tr")
            nc.tensor.transpose(pt, sw_sb[:, ic * P:(ic + 1) * P], ident)
            nc.vector.tensor_copy(out=swT[ic][:, oc * P:(oc + 1) * P], in_=pt)

    # bias broadcast across all 128 partitions (x2 in free dim) via stride-0 DMA
    bias_bc = singles.tile([P, 2, O_DIM], F32)
    bias_rep = bass.AP(tensor=bias.tensor, offset=bias.offset,
                       ap=[[0, P], [0, 2]] + list(bias.ap))
    nc.sync.dma_start(out=bias_bc, in_=bias_rep)

    # ---- main loop: 4 row-tiles (512 rows) per DMA batch.
    # Row permutation: partition p holds SB *consecutive* DRAM rows, so each
    # partition's slice of a batched DMA is one contiguous SB*1KB segment
    # (fewer descriptors per DMA). The same permutation is applied on the
    # output side, so the result lands in the right place.
    SB = 4  # row-tiles per DMA batch
    assert ntiles % SB == 0
    x4 = x.rearrange("(n p s) i -> n p (s i)", p=P, s=SB)
    y4 = y.rearrange("(n p s) o -> n p (s o)", p=P, s=SB)
    for n in range(ntiles // SB):
        x_sb = xpool.tile([P, SB * I_DIM], F32, tag="x")
        nc.sync.dma_start(out=x_sb, in_=x4[n])

        y_sb = ypool.tile([P, SB // 2, 2, O_DIM], F32, tag="y")
        for sp in range(SB // 2):  # psum_y bank holds 2 row-tiles
            py = psum_y.tile([P, 2, O_DIM], F32, tag="py")
            for s2 in range(2):
                s = sp * 2 + s2
                pt = psum_t.tile([P, 2, P], F32, tag="psum_tr")
                for ic in range(2):
                    nc.tensor.transpose(
                        pt[:, ic],
                        x_sb[:, s * I_DIM + ic * P:s * I_DIM + (ic + 1) * P],
                        ident)
                xT = xtpool.tile([P, 2, P], mm_dtype, tag="xT")  # [i, chunk, b]
                nc.scalar.copy(out=xT, in_=pt)
                for ic in range(2):
                    nc.tensor.matmul(py[:, s2], lhsT=xT[:, ic], rhs=swT[ic],
                                     start=(ic == 0), stop=(ic == 1))
            nc.vector.tensor_add(out=y_sb[:, sp], in0=py, in1=bias_bc)
        nc.gpsimd.dma_start(out=y4[n], in_=y_sb)


_CACHE = {}


def _get_nc(b_rows, mm_dtype=F32):
    key = (b_rows, str(mm_dtype))
    if key not in _CACHE:
        _CACHE[key] = build_kernel(b_rows, mm_dtype)
    return _CACHE[key]


def run_sharded(x, W, scale, bias, trace=False, mm_dtype=F32):
    """Run the SPMD kernel on 8 cores; returns (y_full, BassKernelResults)."""
    x = np.ascontiguousarray(x, dtype=np.float32)
    W = np.ascontiguousarray(W, dtype=np.float32)
    scale = np.ascontiguousarray(scale, dtype=np.float32)
    bias = np.ascontiguousarray(bias, dtype=np.float32)
    b_shard = x.shape[0] // N_CORES
    nc = _get_nc(b_shard, mm_dtype)
    xs = x.reshape(N_CORES, b_shard, I_DIM)
    in_maps = [
        {"x": np.ascontiguousarray(xs[c]), "w": W, "scale": scale, "bias": bias}
        for c in range(N_CORES)
    ]
    res = bass_utils.run_bass_kernel_spmd(
        nc, in_maps, core_ids=list(range(N_CORES)), trace=trace,
        trace_cores=list(range(N_CORES)) if trace else None,
    )
    y = np.concatenate([res.results[c]["y"] for c in range(N_CORES)], axis=0)
    return y, res


def kernel(x, W, scale, bias):
    y, _ = run_sharded(x, W, scale, bias, trace=False)
    return y
